# revision 5
# baseline (speedup 1.0000x reference)
"""Trainium2 Bass kernel: per-batch cosine-distance matrix.

out[b] = 1 - metric[b] @ metric[b].T   where metric = x / ||x||_2 (last dim)
x: [32, 1024, 768] f32  ->  out: [32, 1024, 1024] f32

Sharding: data-parallel over batch. 8 cores x 4 batches each; no
cross-core communication.

Design (raw-fp8 gram + scale-at-eviction; v2):
  Host prep: cast x to fp8 e4m3, transpose each batch to xT8 [C, T]
  (layout/dtype prep only - all math runs on device). Per batch, in
  REVERSE row-block order m = 7..0:
    1. DMA xT8 -> SBUF x83 [128, (k, t)] fp8 split across SP+ACT queues.
    2. PE: gram row m on RAW x8: psum[128, W] covering cols m*128..T
       (upper triangle incl. diagonal block). fp8 DoubleRow, K=256 per
       matmul, j-outer so the two width-chunks of a (m, j) pair share
       one LDWEIGHTS (ldweights=False on the second chunk). All PE
       instructions are chained (chain_iter_dep) so the tile scheduler
       cannot reorder them - weight reuse is order-dependent.
    3. Norms come from the row's own diagonal block: DVE stt-vs-identity
       extracts ss -> rv[:, m]; reciprocal; ACT sqrt -> rinvv (bf16,
       per-partition row scale). No separate diag pass.
    4. Per-block rinv chain: PE transpose [128,1]->[1,128] (embedded one
       group later between j=1/j=2 so the chained PE never stalls on the
       DVE/ACT latency), ACT copy with scale=-1 -> row8, DMA to DRAM,
       DMA partition-broadcast back into RI[:, m-block] (bf16, negated).
       Reverse order means row m's eviction needs RI blocks m..7, all of
       which are already broadcast by the time row m's psum is full.
    5. Eviction: DVE tensor_tensor sc = psum * RI  (= -raw*rinv_j, bf16)
       then ACT activation ob = Copy(sc * rinvv[:, m] + 1.0) -> f16
       (per-partition AP scale folds the row normalization and the
       1-minus into the eviction pass). DMA out on alternating queues.
  Host post: upcast f16 -> f32 and mirror the (symmetric) lower half.

vs the met8 baseline (87.3us): removes the met8 elementwise pass
(~64us of DVE+Pool work) and the separate diagonal matmul pass
(24 LDW+MM/batch), and halves LDWEIGHTS count via j-outer reuse.
"""

import sys
import time
from contextlib import ExitStack

_TRN_REPO = "/opt/trn_rl_repo"
if _TRN_REPO not in sys.path:
    sys.path.insert(0, _TRN_REPO)

import numpy as np
import ml_dtypes

import concourse.bacc as bacc
import concourse.mybir as mybir
import concourse.tile as tile
from concourse.bass_utils import run_bass_kernel_spmd
from concourse.masks import make_identity

B, T, C = 32, 1024, 768
N_CORES = 8
BPC = B // N_CORES   # batches per core
KC = C // 128        # 6 k-chunks
KP = KC // 2         # 3 k-pairs (DoubleRow)
TT = T // 128        # 8 row blocks
F32 = mybir.dt.float32
F16 = mybir.dt.float16
BF16 = mybir.dt.bfloat16
F8 = mybir.dt.float8e4
AF = mybir.ActivationFunctionType
ALU = mybir.AluOpType
DR = mybir.MatmulPerfMode.DoubleRow


def build():
    nc = bacc.Bacc("TRN2", target_bir_lowering=False, debug=False,
                   num_devices=N_CORES)
    xT8 = nc.dram_tensor("xT8", [BPC, C, T], F8, kind="ExternalInput").ap()
    out = nc.dram_tensor("out", [BPC, T, T], F16, kind="ExternalOutput").ap()
    rowsc = nc.dram_tensor("rowsc", [BPC, T], BF16, kind="Internal").ap()

    with tile.TileContext(nc) as tc, ExitStack() as ctx:
        x_pool = ctx.enter_context(tc.tile_pool(name="x", bufs=3))
        ri_pool = ctx.enter_context(tc.tile_pool(name="ri", bufs=2))
        s_pool = ctx.enter_context(tc.tile_pool(name="s", bufs=2))
        sc_pool = ctx.enter_context(tc.tile_pool(name="sc", bufs=3))
        ob_pool = ctx.enter_context(tc.tile_pool(name="ob", bufs=4))
        c_pool = ctx.enter_context(tc.tile_pool(name="c", bufs=1))
        psw_pool = ctx.enter_context(
            tc.tile_pool(name="psw", bufs=2, space="PSUM"))  # rows 0-3, 2 banks
        psn_pool = ctx.enter_context(
            tc.tile_pool(name="psn", bufs=3, space="PSUM"))  # rows 4-7, 1 bank
        psT_pool = ctx.enter_context(
            tc.tile_pool(name="psT", bufs=1, space="PSUM"))

        identf = c_pool.tile([128, 128], F32)
        make_identity(nc, identf[:])
        dummy = c_pool.tile([128, 128], F32, tag="dummy")

        # warm the ACT Sqrt table while the first DMA flies
        warm = c_pool.tile([128, 1], F32, tag="warm")
        nc.vector.memset(warm[:], 1.0)
        warm2 = c_pool.tile([128, 1], F32, tag="warm2")
        nc.scalar.sqrt(warm2[:], warm[:])

        x83s, pgs, RIs, rvs, rrs, rivs = {}, {}, {}, {}, {}, {}

        def chain(inst):
            tc.chain_iter_dep("pe_order", inst.ins)

        def emit_load(b):
            x8 = x_pool.tile([128, KC * T], F8, tag="x8", name=f"x8_{b}")
            x83 = x8[:].rearrange("p (k t) -> p k t", k=KC)
            src = xT8[b].rearrange("(k p) t -> p k t", p=128)
            nc.sync.dma_start(x83[:, :KP, :], src[:, :KP, :])
            nc.scalar.dma_start(x83[:, KP:, :], src[:, KP:, :])
            x83s[b] = x83

        def alloc_ri(b):
            RIs[b] = ri_pool.tile([128, T], BF16, tag="RI", name=f"RI_{b}")
            rvs[b] = s_pool.tile([128, TT], F32, tag="rv", name=f"rv_{b}")
            rrs[b] = s_pool.tile([128, TT], F32, tag="rr", name=f"rr_{b}")
            rivs[b] = s_pool.tile([128, TT], F32, tag="riv", name=f"riv_{b}")

        def emit_transpose_chain(b, m):
            # rinvv[:, m] -> [1, 128] -> (negated bf16) -> DRAM -> broadcast
            # back as RI[:, m*128:(m+1)*128]. PE transpose is chained; its
            # rinvv dep is ~1 group old so it does not stall the PE.
            rvT = psT_pool.tile([1, 128], F32, tag="rvT",
                                name=f"rvT_{b}_{m}")
            mmT = nc.tensor.transpose(rvT[:], rivs[b][:, m:m + 1], identf[:])
            chain(mmT)
            row8 = s_pool.tile([1, 128], BF16, tag="row8", bufs=3,
                               name=f"row8_{b}_{m}")
            nc.scalar.activation(row8[:], rvT[:], AF.Copy, bias=0.0,
                                 scale=-1.0)
            sl = slice(m * 128, (m + 1) * 128)
            nc.gpsimd.dma_start(rowsc[b, sl], row8[:])
            nc.gpsimd.dma_start(
                RIs[b][:, sl],
                rowsc[b, sl].unsqueeze(0).to_broadcast((128, 128)))

        def emit_evict(b, m):
            # needs RI blocks m..7 (all broadcast by now, reverse order)
            n0 = m * 128
            W = T - n0
            pg = pgs[(b, m)]
            sc = sc_pool.tile([128, W], BF16, tag="sc", name=f"sc_{b}_{m}",
                              padded_shape=[128, T])
            nc.vector.tensor_tensor(sc[:], pg[:], RIs[b][:, n0:], ALU.mult)
            ob = ob_pool.tile([128, W], F16, tag="ob", name=f"ob_{b}_{m}",
                              padded_shape=[128, T])
            nc.scalar.activation(ob[:], sc[:], AF.Copy, bias=1.0,
                                 scale=rivs[b][:, m:m + 1])
            eng = nc.sync if m % 2 == 0 else nc.scalar
            eng.dma_start(out[b, n0:n0 + 128, n0:], ob[:])

        def pe_group(b, m, embed=None):
            # gram row m on raw x8: psum cols m*128..T, j-outer with
            # LDWEIGHTS shared across the row's width chunks.
            x83 = x83s[b]
            n0 = m * 128
            W = T - n0
            pool = psw_pool if W > 512 else psn_pool
            tag = "pgw" if W > 512 else "pgn"
            pg = pool.tile([128, W], F32, tag=tag, name=f"pg_{b}_{m}",
                           padded_shape=[128, T if W > 512 else 512])
            pgs[(b, m)] = pg
            chunks = [(0, min(512, W))]
            if W > 512:
                chunks.append((512, W - 512))
            msl = slice(n0, n0 + 128)
            for j in range(KP):
                for ci, (off, w) in enumerate(chunks):
                    mm = nc.tensor.matmul(
                        pg[:, off:off + w],
                        x83[:, 2 * j:2 * j + 2, msl],
                        x83[:, 2 * j:2 * j + 2, n0 + off:n0 + off + w],
                        start=(j == 0), stop=(j == KP - 1),
                        perf_mode=DR, skip_group_check=True)
                    if ci > 0:
                        mm.ins.ldweights = False
                    chain(mm)
                if j == 1 and embed is not None:
                    eb, em = embed
                    emit_transpose_chain(eb, em)
                    emit_evict(eb, em)

        def emit_extract(b, m):
            # ss on the diagonal of the row's own diag block -> rinvv[:, m]
            pg = pgs[(b, m)]
            nc.vector.scalar_tensor_tensor(
                dummy[:], pg[:, 0:128], 1.0, identf[:], ALU.mult, ALU.mult,
                accum_out=rvs[b][:, m:m + 1])
            nc.vector.reciprocal(rrs[b][:, m:m + 1], rvs[b][:, m:m + 1])
            nc.scalar.activation(rivs[b][:, m:m + 1], rrs[b][:, m:m + 1],
                                 AF.Sqrt, bias=0.0, scale=1.0)

        # ---- pipeline ----
        emit_load(0)
        if BPC > 1:
            emit_load(1)
        prev = None
        for b in range(BPC):
            alloc_ri(b)
            for m in range(TT - 1, -1, -1):
                if b + 2 < BPC and m == 5:
                    emit_load(b + 2)
                pe_group(b, m, embed=prev)
                emit_extract(b, m)
                prev = (b, m)
        # drain the last block's chain + eviction
        eb, em = prev
        emit_transpose_chain(eb, em)
        emit_evict(eb, em)

    nc.compile()
    return nc


_MIRROR_MASK = None


def host_post(upper_f16):
    """Mirror the upper triangle onto the (unwritten) lower half, f32."""
    global _MIRROR_MASK
    if _MIRROR_MASK is None:
        idx = np.arange(T)
        _MIRROR_MASK = (idx[None, :] >= idx[:, None])[None]  # j >= i
    u = upper_f16.astype(np.float32)
    return np.where(_MIRROR_MASK, u, u.transpose(0, 2, 1))


def host_prep(x):
    x = np.asarray(x)
    x8 = x.astype(ml_dtypes.float8_e4m3)               # [B, T, C]
    xT8 = np.ascontiguousarray(x8.transpose(0, 2, 1))  # [B, C, T]
    return xT8


def run(x, trace=False):
    nc = build()
    xT8 = host_prep(x)
    in_maps = [{"xT8": xT8[i * BPC:(i + 1) * BPC]} for i in range(N_CORES)]
    last_err = None
    for _attempt in range(3):
        try:
            res = run_bass_kernel_spmd(nc, in_maps, list(range(N_CORES)),
                                       trace=trace)
            break
        except Exception as e:  # transient device wedge: retry
            last_err = e
            time.sleep(2.0)
    else:
        raise last_err
    out = np.concatenate([host_post(res.results[i]["out"])
                          for i in range(N_CORES)], axis=0)
    return out, res


def kernel(x):
    out, _ = run(x, trace=False)
    return out


# revision 6
# speedup vs baseline: 1.5989x; 1.5989x over previous
"""Trainium2 Bass kernel: per-batch cosine-distance matrix.

out[b] = 1 - metric[b] @ metric[b].T   where metric = x / ||x||_2 (last dim)
x: [32, 1024, 768] f32  ->  out: [32, 1024, 1024] f32

Sharding: data-parallel over batch. 8 cores x 4 batches each; no
cross-core communication.

Design (raw-fp8 gram + scale-at-eviction; v3):
  Host prep: cast x to fp8 e4m3, transpose each batch to xT8 [C, T]
  (layout/dtype prep only - all math runs on device). Per batch, in
  REVERSE row-block order m = 7..0:
    1. DMA xT8 -> SBUF x83 [128, (k, t)] fp8 split across SP+ACT queues.
    2. PE: gram row m on RAW x8 (no met8 prescale): psum chunks of
       <=512 f32 covering cols m*128..T. fp8 DoubleRow K=256/matmul,
       j-outer so all chunks of a (m, j) pair share one LDWEIGHTS.
       The tile legalizer emits one LDWEIGHTS per matmul regardless,
       so dedup_ldweights() below strips redundant loads post-legalize
       (migrating their waits/deps), enabled by chaining every PE
       instruction (chain_iter_dep) so the final PE order is exactly
       emission order.
    3. Norms from the row's own diagonal 128-block: DVE stt-vs-identity
       -> rv[:, m]; DVE reciprocal; ACT sqrt -> rinvv (f32 row scale).
    4. rinv broadcast chain, fully off the PE: Pool negate into
       rivn32[:, m], DVE 32x32 stream-transpose of rivn32 -> rvTd,
       SWDGE (Pool) DMA rvTd[m::32, :] -> DRAM rowsc (contiguous f32),
       SWDGE broadcast back into RI[:, m-block] (-rinv, f32). Reverse
       order means row m's eviction needs only blocks m..7 of RI, all
       broadcast by the time row m's psum is complete.
    5. Eviction per chunk: DVE tensor_tensor sc = psum * RI (f32,
       = -raw*rinv_j), ACT ob = Copy(sc * rinvv[:, m] + 1.0) -> f16
       (per-partition AP scale), DMA out on alternating SP/ACT queues.
  All psum tiles are single-bank [128, <=512] chunks from one bufs=8
  pool, so slot reuse is fine-grained and the chain latency (~2-4us)
  is covered by ~7 live chunks.
  Host post: upcast f16 -> f32 and mirror the (symmetric) lower half.
"""

import sys
import time
from contextlib import ExitStack

_TRN_REPO = "/opt/trn_rl_repo"
if _TRN_REPO not in sys.path:
    sys.path.insert(0, _TRN_REPO)

import numpy as np
import ml_dtypes

import concourse.bacc as bacc
import concourse.mybir as mybir
import concourse.tile as tile
from concourse.bass_utils import run_bass_kernel_spmd
from concourse.masks import make_identity

B, T, C = 32, 1024, 768
N_CORES = 8
BPC = B // N_CORES   # batches per core
KC = C // 128        # 6 k-chunks
KP = KC // 2         # 3 k-pairs (DoubleRow)
TT = T // 128        # 8 row blocks
F32 = mybir.dt.float32
F16 = mybir.dt.float16
BF16 = mybir.dt.bfloat16
F8 = mybir.dt.float8e4
AF = mybir.ActivationFunctionType
ALU = mybir.AluOpType
DR = mybir.MatmulPerfMode.DoubleRow


def dedup_ldweights(nc):
    """Remove InstLdweights whose weights AP equals the currently-loaded
    stationary (set by the previous LDW and not clobbered since). Runs
    after tile legalization (final instruction order) and before
    nc.compile() (semaphore generation), so migrating the removed LDW's
    sync_info and dependency edges onto the following matmul is safe.
    """
    removed = 0
    for f in nc.m.functions:
        for bb in f.blocks:
            keep = []
            last_key = None
            donors = []
            for inst in bb.instructions:
                if getattr(inst, "engine", None) != mybir.EngineType.PE:
                    keep.append(inst)
                    continue
                if isinstance(inst, mybir.InstLdweights):
                    try:
                        key = (str(inst.ins[0]), str(inst.perf_mode))
                    except Exception:
                        key = None
                    if key is not None and key == last_key:
                        donors.append(inst)
                        removed += 1
                        continue
                    last_key = key
                    keep.append(inst)
                else:
                    if isinstance(inst, mybir.InstMatmult) and inst.is_transpose:
                        last_key = None
                    elif not isinstance(inst, mybir.InstMatmult):
                        last_key = None
                    for d in donors:
                        inst.merge_dependencies_from(d)
                        dsi = d.sync_info
                        if dsi is not None and (dsi.on_wait or dsi.on_update):
                            si = inst.sync_info
                            ow = list(si.on_wait) if si else []
                            ou = list(si.on_update) if si else []
                            inst.sync_info = mybir.SyncInfo(
                                on_wait=ow + list(dsi.on_wait),
                                on_update=ou + list(dsi.on_update))
                    donors = []
                    keep.append(inst)
            assert not donors, "trailing removed LDW with no successor"
            bb.instructions[:] = keep
    return removed


def build():
    nc = bacc.Bacc("TRN2", target_bir_lowering=False, debug=False,
                   num_devices=N_CORES)
    xT8 = nc.dram_tensor("xT8", [BPC, C, T], F8, kind="ExternalInput").ap()
    out = nc.dram_tensor("out", [BPC, T, T], F16, kind="ExternalOutput").ap()
    rowsc = nc.dram_tensor("rowsc", [BPC, T], F32, kind="Internal").ap()

    with tile.TileContext(nc) as tc, ExitStack() as ctx:
        x_pool = ctx.enter_context(tc.tile_pool(name="x", bufs=3))
        ri_pool = ctx.enter_context(tc.tile_pool(name="ri", bufs=2))
        s_pool = ctx.enter_context(tc.tile_pool(name="s", bufs=2))
        sc_pool = ctx.enter_context(tc.tile_pool(name="sc", bufs=4))
        ob_pool = ctx.enter_context(tc.tile_pool(name="ob", bufs=6))
        c_pool = ctx.enter_context(tc.tile_pool(name="c", bufs=1))
        ps_pool = ctx.enter_context(
            tc.tile_pool(name="ps", bufs=8, space="PSUM"))  # 1 bank/slot

        identf = c_pool.tile([128, 128], F32)
        make_identity(nc, identf[:])
        dummy = c_pool.tile([128, 128], F32, tag="dummy")

        # warm the ACT Sqrt table while the first DMA flies
        warm = c_pool.tile([128, 1], F32, tag="warm")
        nc.vector.memset(warm[:], 1.0)
        warm2 = c_pool.tile([128, 1], F32, tag="warm2")
        nc.scalar.sqrt(warm2[:], warm[:])

        x83s, pgs, RIs, rvs, rrs, rivs, rivn32s = {}, {}, {}, {}, {}, {}, {}

        def chain(inst):
            tc.chain_iter_dep("pe_order", inst.ins)

        def emit_load(b):
            x8 = x_pool.tile([128, KC * T], F8, tag="x8", name=f"x8_{b}")
            x83 = x8[:].rearrange("p (k t) -> p k t", k=KC)
            src = xT8[b].rearrange("(k p) t -> p k t", p=128)
            nc.sync.dma_start(x83[:, :KP, :], src[:, :KP, :])
            nc.scalar.dma_start(x83[:, KP:, :], src[:, KP:, :])
            x83s[b] = x83

        def alloc_ri(b):
            RIs[b] = ri_pool.tile([128, T], F32, tag="RI", name=f"RI_{b}")
            rvs[b] = s_pool.tile([128, TT], F32, tag="rv", name=f"rv_{b}")
            rrs[b] = s_pool.tile([128, TT], F32, tag="rr", name=f"rr_{b}")
            rivs[b] = s_pool.tile([128, TT], F32, tag="riv", name=f"riv_{b}")
            rivn32s[b] = s_pool.tile([128, 32], F32, tag="rivn32",
                                     name=f"rivn32_{b}")
            nc.gpsimd.memset(rivn32s[b][:], 0.0)

        def chunks_of(m):
            W = T - m * 128
            ch = [(0, min(512, W))]
            if W > 512:
                ch.append((512, W - 512))
            return ch

        def pe_group(b, m):
            # gram row m on raw x8, j-outer; chunk tiles are 1 psum bank
            x83 = x83s[b]
            n0 = m * 128
            tiles = []
            for ci, (off, w) in enumerate(chunks_of(m)):
                pgt = ps_pool.tile([128, w], F32, tag="pg",
                                   name=f"pg_{b}_{m}_{ci}",
                                   padded_shape=[128, 512])
                tiles.append((pgt, off, w))
            pgs[(b, m)] = tiles
            msl = slice(n0, n0 + 128)
            for j in range(KP):
                for pgt, off, w in tiles:
                    mm = nc.tensor.matmul(
                        pgt[:],
                        x83[:, 2 * j:2 * j + 2, msl],
                        x83[:, 2 * j:2 * j + 2, n0 + off:n0 + off + w],
                        start=(j == 0), stop=(j == KP - 1),
                        perf_mode=DR, skip_group_check=True)
                    chain(mm)

        def emit_extract(b, m):
            # ss from the diagonal of chunk 0 -> rinvv[:, m]; then the
            # negated copy for the broadcast chain (Pool, off DVE/ACT).
            pgt = pgs[(b, m)][0][0]
            nc.vector.scalar_tensor_tensor(
                dummy[:], pgt[:, 0:128], 1.0, identf[:], ALU.mult, ALU.mult,
                accum_out=rvs[b][:, m:m + 1])
            nc.vector.reciprocal(rrs[b][:, m:m + 1], rvs[b][:, m:m + 1])
            nc.scalar.activation(rivs[b][:, m:m + 1], rrs[b][:, m:m + 1],
                                 AF.Sqrt, bias=0.0, scale=1.0)
            nc.gpsimd.tensor_scalar_mul(rivn32s[b][:, m:m + 1],
                                        rivs[b][:, m:m + 1], -1.0)

        def emit_chain(b, m):
            # DVE 32x32 stream transpose: col m of rivn32 lands on
            # partitions m, 32+m, 64+m, 96+m (32 cols each) -> contiguous
            # DRAM row -> partition-broadcast into RI[:, m-block].
            rvTd = s_pool.tile([128, 32], F32, tag="rvTd", bufs=3,
                               name=f"rvTd_{b}_{m}")
            nc.vector.transpose(rvTd[:], rivn32s[b][:])
            sl = slice(m * 128, (m + 1) * 128)
            nc.gpsimd.dma_start(rowsc[b, sl], rvTd[m::32, :])
            nc.gpsimd.dma_start(
                RIs[b][:, sl],
                rowsc[b, sl].unsqueeze(0).to_broadcast((128, 128)))

        def emit_evict(b, m):
            # needs RI blocks m..7 (all broadcast; reverse order)
            n0 = m * 128
            for ci, (pgt, off, w) in enumerate(pgs[(b, m)]):
                sc = sc_pool.tile([128, w], F32, tag="sc",
                                  name=f"sc_{b}_{m}_{ci}",
                                  padded_shape=[128, 512])
                nc.vector.tensor_tensor(
                    sc[:], pgt[:], RIs[b][:, n0 + off:n0 + off + w], ALU.mult)
                ob = ob_pool.tile([128, w], F16, tag="ob",
                                  name=f"ob_{b}_{m}_{ci}",
                                  padded_shape=[128, 512])
                nc.scalar.activation(ob[:], sc[:], AF.Copy, bias=1.0,
                                     scale=rivs[b][:, m:m + 1])
                eng = nc.sync if (m + ci) % 2 == 0 else nc.scalar
                eng.dma_start(out[b, n0:n0 + 128, n0 + off:n0 + off + w],
                              ob[:])

        # ---- pipeline ----
        emit_load(0)
        if BPC > 1:
            emit_load(1)
        prev = None
        for b in range(BPC):
            alloc_ri(b)
            for m in range(TT - 1, -1, -1):
                if b + 2 < BPC and m == 5:
                    emit_load(b + 2)
                pe_group(b, m)
                emit_extract(b, m)
                emit_chain(b, m)
                if prev is not None:
                    emit_evict(*prev)
                prev = (b, m)
        emit_evict(*prev)

    n = dedup_ldweights(nc)
    assert n > 0, "expected to remove redundant LDWEIGHTS"
    nc.compile()
    return nc


_MIRROR_MASK = None


def host_post(upper_f16):
    """Mirror the upper triangle onto the (unwritten) lower half, f32."""
    global _MIRROR_MASK
    if _MIRROR_MASK is None:
        idx = np.arange(T)
        _MIRROR_MASK = (idx[None, :] >= idx[:, None])[None]  # j >= i
    u = upper_f16.astype(np.float32)
    return np.where(_MIRROR_MASK, u, u.transpose(0, 2, 1))


def host_prep(x):
    x = np.asarray(x)
    x8 = x.astype(ml_dtypes.float8_e4m3)               # [B, T, C]
    xT8 = np.ascontiguousarray(x8.transpose(0, 2, 1))  # [B, C, T]
    return xT8


def run(x, trace=False):
    nc = build()
    xT8 = host_prep(x)
    in_maps = [{"xT8": xT8[i * BPC:(i + 1) * BPC]} for i in range(N_CORES)]
    last_err = None
    for _attempt in range(3):
        try:
            res = run_bass_kernel_spmd(nc, in_maps, list(range(N_CORES)),
                                       trace=trace)
            break
        except Exception as e:  # transient device wedge: retry
            last_err = e
            time.sleep(2.0)
    else:
        raise last_err
    out = np.concatenate([host_post(res.results[i]["out"])
                          for i in range(N_CORES)], axis=0)
    return out, res


def kernel(x):
    out, _ = run(x, trace=False)
    return out


# revision 7
# speedup vs baseline: 2.1640x; 1.3534x over previous
"""Trainium2 Bass kernel: per-batch cosine-distance matrix.

out[b] = 1 - metric[b] @ metric[b].T   where metric = x / ||x||_2 (last dim)
x: [32, 1024, 768] f32  ->  out: [32, 1024, 1024] f32

Sharding: data-parallel over batch. 8 cores x 4 batches each; no
cross-core communication.

Design (raw-fp8 gram, SBUF-staged scale-at-eviction; v4):
  Host prep: cast x to fp8 e4m3, transpose each batch to xT8 [C, T]
  (layout/dtype prep only - all math runs on device). Per batch,
  row blocks m = 0..7 (upper triangle incl. diagonal, cols m*128..T):
    1. DMA xT8 -> SBUF x83 [128, (k, t)] fp8, 6 chunked DMAs over the
       SP+ACT hwdge queues.
    2. PE: gram row m on RAW x8 (no met8 prescale): psum chunks of
       <=512 f32. fp8 DoubleRow K=256/matmul, j-outer so the chunks of
       a (m, j) pair share one LDWEIGHTS: the tile legalizer emits one
       LDWEIGHTS per matmul, so dedup_ldweights() strips the redundant
       loads post-legalize (migrating waits/deps); every PE instruction
       is chained (chain_iter_dep) so the final PE order is exactly
       emission order, making the weight reuse sound.
    3. Each psum chunk is immediately staged to SBUF bf16 (DVE
       tensor_copy / ACT copy split) into raw[b] [128, 4608] - psum
       slots recycle within ~1us, so the PE never waits on the
       normalization chain. 1-bank chunk slots, bufs=6.
    4. Norms: DVE stt-vs-identity on the staged diagonal block
       -> rv[:, m] (=ss, and since eviction reads the SAME bf16 values,
       the output diagonal is exactly 0). Once per batch: DVE
       reciprocal, ACT sqrt -> rinvv f32 [128, 8]; PE transpose
       [128,8]->[8,128] (chained one group into the next batch); ACT
       copy with scale=-1 -> bf16 row; DMA to DRAM; DMA
       partition-broadcast back -> RI [128, T] bf16 (= -rinv).
    5. Evictions of batch b run during batch b+1 (one row per
       iteration): DVE tensor_tensor sc = raw * RI (all-bf16 SBUF, 2x
       mode), ACT ob = Copy(sc * rinvv[:, m] + 1.0) -> f16
       (per-partition AP scale), DMA out on alternating queues.
  Host post: upcast f16 -> f32 and mirror the (symmetric) lower half.
"""

import sys
import time
from contextlib import ExitStack

_TRN_REPO = "/opt/trn_rl_repo"
if _TRN_REPO not in sys.path:
    sys.path.insert(0, _TRN_REPO)

import numpy as np
import ml_dtypes

import concourse.bacc as bacc
import concourse.mybir as mybir
import concourse.tile as tile
from concourse.bass_utils import run_bass_kernel_spmd
from concourse.masks import make_identity

B, T, C = 32, 1024, 768
N_CORES = 8
BPC = B // N_CORES   # batches per core
KC = C // 128        # 6 k-chunks
KP = KC // 2         # 3 k-pairs (DoubleRow)
TT = T // 128        # 8 row blocks
F32 = mybir.dt.float32
F16 = mybir.dt.float16
BF16 = mybir.dt.bfloat16
F8 = mybir.dt.float8e4
AF = mybir.ActivationFunctionType
ALU = mybir.AluOpType
DR = mybir.MatmulPerfMode.DoubleRow

ROW_OFF = [0]
for _m in range(TT):
    ROW_OFF.append(ROW_OFF[-1] + (T - _m * 128))  # prefix offsets into raw


def dedup_ldweights(nc):
    """Remove InstLdweights whose weights AP equals the currently-loaded
    stationary (set by the previous LDW and not clobbered since). Runs
    after tile legalization (final instruction order) and before
    nc.compile() (semaphore generation), so migrating the removed LDW's
    sync_info and dependency edges onto the following matmul is safe.
    """
    removed = 0
    for f in nc.m.functions:
        for bb in f.blocks:
            keep = []
            last_key = None
            donors = []
            for inst in bb.instructions:
                if getattr(inst, "engine", None) != mybir.EngineType.PE:
                    keep.append(inst)
                    continue
                if isinstance(inst, mybir.InstLdweights):
                    try:
                        key = (str(inst.ins[0]), str(inst.perf_mode))
                    except Exception:
                        key = None
                    if key is not None and key == last_key:
                        donors.append(inst)
                        removed += 1
                        continue
                    last_key = key
                    keep.append(inst)
                else:
                    if isinstance(inst, mybir.InstMatmult) and inst.is_transpose:
                        last_key = None
                    elif not isinstance(inst, mybir.InstMatmult):
                        last_key = None
                    for d in donors:
                        inst.merge_dependencies_from(d)
                        dsi = d.sync_info
                        if dsi is not None and (dsi.on_wait or dsi.on_update):
                            si = inst.sync_info
                            ow = list(si.on_wait) if si else []
                            ou = list(si.on_update) if si else []
                            inst.sync_info = mybir.SyncInfo(
                                on_wait=ow + list(dsi.on_wait),
                                on_update=ou + list(dsi.on_update))
                    donors = []
                    keep.append(inst)
            assert not donors, "trailing removed LDW with no successor"
            bb.instructions[:] = keep
    return removed


def build():
    nc = bacc.Bacc("TRN2", target_bir_lowering=False, debug=False,
                   num_devices=N_CORES)
    xT8 = nc.dram_tensor("xT8", [BPC, C, T], F8, kind="ExternalInput").ap()
    out = nc.dram_tensor("out", [BPC, T, T], F16, kind="ExternalOutput").ap()
    rowsc = nc.dram_tensor("rowsc", [BPC, T], BF16, kind="Internal").ap()

    with tile.TileContext(nc) as tc, ExitStack() as ctx:
        x_pool = ctx.enter_context(tc.tile_pool(name="x", bufs=3))
        raw_pool = ctx.enter_context(tc.tile_pool(name="raw", bufs=2))
        ri_pool = ctx.enter_context(tc.tile_pool(name="ri", bufs=2))
        s_pool = ctx.enter_context(tc.tile_pool(name="s", bufs=2))
        sc_pool = ctx.enter_context(tc.tile_pool(name="sc", bufs=3))
        ob_pool = ctx.enter_context(tc.tile_pool(name="ob", bufs=4))
        c_pool = ctx.enter_context(tc.tile_pool(name="c", bufs=1))
        ps_pool = ctx.enter_context(
            tc.tile_pool(name="ps", bufs=6, space="PSUM"))  # 1 bank/slot
        psT_pool = ctx.enter_context(
            tc.tile_pool(name="psT", bufs=2, space="PSUM"))

        identf = c_pool.tile([128, 128], F32)
        make_identity(nc, identf[:])
        identb = c_pool.tile([128, 128], BF16, tag="identb")
        make_identity(nc, identb[:])
        dummyb = c_pool.tile([128, 128], BF16, tag="dummyb")

        # warm the ACT Sqrt table while the first DMA flies
        warm = c_pool.tile([128, 1], F32, tag="warm")
        nc.vector.memset(warm[:], 1.0)
        warm2 = c_pool.tile([128, 1], F32, tag="warm2")
        nc.scalar.sqrt(warm2[:], warm[:])

        x83s, raws, RIs, rvs, rrs, rivs, rvTs = {}, {}, {}, {}, {}, {}, {}

        def chain(inst):
            tc.chain_iter_dep("pe_order", inst.ins)

        def emit_load(b):
            x8 = x_pool.tile([128, KC * T], F8, tag="x8", name=f"x8_{b}")
            x83 = x8[:].rearrange("p (k t) -> p k t", k=KC)
            src = xT8[b].rearrange("(k p) t -> p k t", p=128)
            for k in range(KC):
                eng = nc.sync if k < KP else nc.scalar
                eng.dma_start(x83[:, k, :], src[:, k, :])
            x83s[b] = x83

        def alloc_batch(b):
            raws[b] = raw_pool.tile([128, ROW_OFF[TT]], BF16, tag="raw",
                                    name=f"raw_{b}")
            RIs[b] = ri_pool.tile([128, T], BF16, tag="RI", name=f"RI_{b}")
            rvs[b] = s_pool.tile([128, TT], F32, tag="rv", name=f"rv_{b}")
            rrs[b] = s_pool.tile([128, TT], F32, tag="rr", name=f"rr_{b}")
            rivs[b] = s_pool.tile([128, TT], F32, tag="riv", name=f"riv_{b}")

        def chunks_of(m):
            W = T - m * 128
            ch = [(0, min(512, W))]
            if W > 512:
                ch.append((512, W - 512))
            return ch

        def pe_group(b, m):
            # gram row m on raw x8, j-outer; chunk tiles are 1 psum bank;
            # each chunk staged to SBUF bf16 right after its stop matmul.
            x83 = x83s[b]
            n0 = m * 128
            tiles = []
            for ci, (off, w) in enumerate(chunks_of(m)):
                pgt = ps_pool.tile([128, w], F32, tag="pg",
                                   name=f"pg_{b}_{m}_{ci}",
                                   padded_shape=[128, 512])
                tiles.append((pgt, off, w))
            msl = slice(n0, n0 + 128)
            for j in range(KP):
                for pgt, off, w in tiles:
                    mm = nc.tensor.matmul(
                        pgt[:],
                        x83[:, 2 * j:2 * j + 2, msl],
                        x83[:, 2 * j:2 * j + 2, n0 + off:n0 + off + w],
                        start=(j == 0), stop=(j == KP - 1),
                        perf_mode=DR, skip_group_check=True)
                    chain(mm)
            # stage to SBUF: narrow rows via ACT, wide chunks via DVE
            for pgt, off, w in tiles:
                dst = raws[b][:, ROW_OFF[m] + off:ROW_OFF[m] + off + w]
                if m >= 4:
                    nc.scalar.activation(dst, pgt[:], AF.Copy, bias=0.0,
                                         scale=1.0)
                else:
                    nc.vector.tensor_copy(dst, pgt[:])
            # ss from the staged diagonal block (same values eviction uses)
            nc.vector.scalar_tensor_tensor(
                dummyb[:], raws[b][:, ROW_OFF[m]:ROW_OFF[m] + 128], 1.0,
                identb[:], ALU.mult, ALU.mult,
                accum_out=rvs[b][:, m:m + 1])

        def emit_norm_head(b):
            # once per batch, right after the last extract
            nc.vector.reciprocal(rrs[b][:], rvs[b][:])
            nc.scalar.activation(rivs[b][:], rrs[b][:], AF.Sqrt, bias=0.0,
                                 scale=1.0)

        def emit_norm_tail(b):
            # PE transpose is emitted by the caller (chained); this is the
            # rest: negated bf16 row, DRAM round trip, partition broadcast.
            row8 = s_pool.tile([TT, 128], BF16, tag="row8",
                               name=f"row8_{b}")
            nc.scalar.activation(row8[:], rvTs[b][:], AF.Copy, bias=0.0,
                                 scale=-1.0)
            nc.sync.dma_start(rowsc[b], row8[:])
            nc.scalar.dma_start(
                RIs[b][:], rowsc[b].unsqueeze(0).to_broadcast((128, T)))

        def emit_transpose(b):
            rvT = psT_pool.tile([TT, 128], F32, tag="rvT", name=f"rvT_{b}")
            mmT = nc.tensor.transpose(rvT[:], rivs[b][:], identf[:])
            chain(mmT)
            rvTs[b] = rvT

        def emit_evict(b, m):
            # sc = raw * RI  (all-bf16 SBUF), ob = 1 + sc*rinv_i -> f16
            n0 = m * 128
            W = T - n0
            sc = sc_pool.tile([128, W], BF16, tag="sc", name=f"sc_{b}_{m}",
                              padded_shape=[128, T])
            nc.vector.tensor_tensor(
                sc[:], raws[b][:, ROW_OFF[m]:ROW_OFF[m] + W],
                RIs[b][:, n0:], ALU.mult)
            ob = ob_pool.tile([128, W], F16, tag="ob", name=f"ob_{b}_{m}",
                              padded_shape=[128, T])
            nc.scalar.activation(ob[:], sc[:], AF.Copy, bias=1.0,
                                 scale=rivs[b][:, m:m + 1])
            eng = nc.sync if m % 2 == 0 else nc.scalar
            eng.dma_start(out[b, n0:n0 + 128, n0:], ob[:])

        # ---- pipeline ----
        emit_load(0)
        if BPC > 1:
            emit_load(1)
        for b in range(BPC):
            alloc_batch(b)
            for m in range(TT):
                if b + 2 < BPC and m == 5:
                    emit_load(b + 2)
                pe_group(b, m)
                if m == TT - 1:
                    emit_norm_head(b)
                if b > 0:
                    if m == 0:
                        emit_transpose(b - 1)
                        emit_norm_tail(b - 1)
                    emit_evict(b - 1, m)
        # epilogue: last batch's chain + evictions
        b = BPC - 1
        emit_transpose(b)
        emit_norm_tail(b)
        for m in range(TT):
            emit_evict(b, m)

    n = dedup_ldweights(nc)
    assert n > 0, "expected to remove redundant LDWEIGHTS"
    nc.compile()
    return nc


_MIRROR_MASK = None


def host_post(upper_f16):
    """Mirror the upper triangle onto the (unwritten) lower half, f32."""
    global _MIRROR_MASK
    if _MIRROR_MASK is None:
        idx = np.arange(T)
        _MIRROR_MASK = (idx[None, :] >= idx[:, None])[None]  # j >= i
    u = upper_f16.astype(np.float32)
    return np.where(_MIRROR_MASK, u, u.transpose(0, 2, 1))


def host_prep(x):
    x = np.asarray(x)
    x8 = x.astype(ml_dtypes.float8_e4m3)               # [B, T, C]
    xT8 = np.ascontiguousarray(x8.transpose(0, 2, 1))  # [B, C, T]
    return xT8


def run(x, trace=False):
    nc = build()
    xT8 = host_prep(x)
    in_maps = [{"xT8": xT8[i * BPC:(i + 1) * BPC]} for i in range(N_CORES)]
    last_err = None
    for _attempt in range(3):
        try:
            res = run_bass_kernel_spmd(nc, in_maps, list(range(N_CORES)),
                                       trace=trace)
            break
        except Exception as e:  # transient device wedge: retry
            last_err = e
            time.sleep(2.0)
    else:
        raise last_err
    out = np.concatenate([host_post(res.results[i]["out"])
                          for i in range(N_CORES)], axis=0)
    return out, res


def kernel(x):
    out, _ = run(x, trace=False)
    return out


# revision 8
# speedup vs baseline: 2.3990x; 1.1086x over previous
"""Trainium2 Bass kernel: per-batch cosine-distance matrix.

out[b] = 1 - metric[b] @ metric[b].T   where metric = x / ||x||_2 (last dim)
x: [32, 1024, 768] f32  ->  out: [32, 1024, 1024] f32

Sharding: data-parallel over batch. 8 cores x 4 batches each; no
cross-core communication.

Design (raw-fp8 gram, SBUF-staged scale-at-eviction; v4):
  Host prep: cast x to fp8 e4m3, transpose each batch to xT8 [C, T]
  (layout/dtype prep only - all math runs on device). Per batch,
  row blocks m = 0..7 (upper triangle incl. diagonal, cols m*128..T):
    1. DMA xT8 -> SBUF x83 [128, (k, t)] fp8, 6 chunked DMAs over the
       SP+ACT hwdge queues.
    2. PE: gram row m on RAW x8 (no met8 prescale): psum chunks of
       <=512 f32. fp8 DoubleRow K=256/matmul, j-outer so the chunks of
       a (m, j) pair share one LDWEIGHTS: the tile legalizer emits one
       LDWEIGHTS per matmul, so dedup_ldweights() strips the redundant
       loads post-legalize (migrating waits/deps); every PE instruction
       is chained (chain_iter_dep) so the final PE order is exactly
       emission order, making the weight reuse sound.
    3. Each psum chunk is immediately staged to SBUF bf16 (DVE
       tensor_copy / ACT copy split) into raw[b] [128, 4608] - psum
       slots recycle within ~1us, so the PE never waits on the
       normalization chain. 1-bank chunk slots, bufs=6.
    4. Norms: DVE stt-vs-identity on the staged diagonal block
       -> rv[:, m] (=ss, and since eviction reads the SAME bf16 values,
       the output diagonal is exactly 0). Once per batch: DVE
       reciprocal, ACT sqrt -> rinvv f32 [128, 8]; PE transpose
       [128,8]->[8,128] (chained one group into the next batch); ACT
       copy with scale=-1 -> bf16 row; DMA to DRAM; DMA
       partition-broadcast back -> RI [128, T] bf16 (= -rinv).
    5. Evictions of batch b run during batch b+1 (one row per
       iteration): DVE tensor_tensor sc = raw * RI (all-bf16 SBUF, 2x
       mode), ACT ob = Copy(sc * rinvv[:, m] + 1.0) -> f16
       (per-partition AP scale), DMA out on alternating queues.
  Host post: upcast f16 -> f32 and mirror the (symmetric) lower half.
"""

import sys
import time
from contextlib import ExitStack

_TRN_REPO = "/opt/trn_rl_repo"
if _TRN_REPO not in sys.path:
    sys.path.insert(0, _TRN_REPO)

import numpy as np
import ml_dtypes

import concourse.bacc as bacc
import concourse.mybir as mybir
import concourse.tile as tile
from concourse.bass_utils import run_bass_kernel_spmd
from concourse.masks import make_identity

B, T, C = 32, 1024, 768
N_CORES = 8
BPC = B // N_CORES   # batches per core
KC = C // 128        # 6 k-chunks
KP = KC // 2         # 3 k-pairs (DoubleRow)
TT = T // 128        # 8 row blocks
F32 = mybir.dt.float32
F16 = mybir.dt.float16
BF16 = mybir.dt.bfloat16
F8 = mybir.dt.float8e4
AF = mybir.ActivationFunctionType
ALU = mybir.AluOpType
DR = mybir.MatmulPerfMode.DoubleRow

ROW_OFF = [0]
for _m in range(TT):
    ROW_OFF.append(ROW_OFF[-1] + (T - _m * 128))  # prefix offsets into raw


def dedup_ldweights(nc):
    """Remove InstLdweights whose weights AP equals the currently-loaded
    stationary (set by the previous LDW and not clobbered since). Runs
    after tile legalization (final instruction order) and before
    nc.compile() (semaphore generation), so migrating the removed LDW's
    sync_info and dependency edges onto the following matmul is safe.
    """
    removed = 0
    for f in nc.m.functions:
        for bb in f.blocks:
            keep = []
            last_key = None
            donors = []
            for inst in bb.instructions:
                if getattr(inst, "engine", None) != mybir.EngineType.PE:
                    keep.append(inst)
                    continue
                if isinstance(inst, mybir.InstLdweights):
                    try:
                        key = (str(inst.ins[0]), str(inst.perf_mode))
                    except Exception:
                        key = None
                    if key is not None and key == last_key:
                        donors.append(inst)
                        removed += 1
                        continue
                    last_key = key
                    keep.append(inst)
                else:
                    if isinstance(inst, mybir.InstMatmult) and inst.is_transpose:
                        last_key = None
                    elif not isinstance(inst, mybir.InstMatmult):
                        last_key = None
                    for d in donors:
                        inst.merge_dependencies_from(d)
                        dsi = d.sync_info
                        if dsi is not None and (dsi.on_wait or dsi.on_update):
                            si = inst.sync_info
                            ow = list(si.on_wait) if si else []
                            ou = list(si.on_update) if si else []
                            inst.sync_info = mybir.SyncInfo(
                                on_wait=ow + list(dsi.on_wait),
                                on_update=ou + list(dsi.on_update))
                    donors = []
                    keep.append(inst)
            assert not donors, "trailing removed LDW with no successor"
            bb.instructions[:] = keep
    return removed


def build():
    nc = bacc.Bacc("TRN2", target_bir_lowering=False, debug=False,
                   num_devices=N_CORES)
    xT8 = nc.dram_tensor("xT8", [BPC, C, T], F8, kind="ExternalInput").ap()
    out = nc.dram_tensor("out", [BPC, T, T], F16, kind="ExternalOutput").ap()
    rowsc = nc.dram_tensor("rowsc", [BPC, T], BF16, kind="Internal").ap()

    with tile.TileContext(nc) as tc, ExitStack() as ctx:
        x_pool = ctx.enter_context(tc.tile_pool(name="x", bufs=3))
        raw_pool = ctx.enter_context(tc.tile_pool(name="raw", bufs=2))
        ri_pool = ctx.enter_context(tc.tile_pool(name="ri", bufs=2))
        s_pool = ctx.enter_context(tc.tile_pool(name="s", bufs=2))
        sc_pool = ctx.enter_context(tc.tile_pool(name="sc", bufs=3))
        ob_pool = ctx.enter_context(tc.tile_pool(name="ob", bufs=4))
        c_pool = ctx.enter_context(tc.tile_pool(name="c", bufs=1))
        ps_pool = ctx.enter_context(
            tc.tile_pool(name="ps", bufs=6, space="PSUM"))  # 1 bank/slot
        psT_pool = ctx.enter_context(
            tc.tile_pool(name="psT", bufs=2, space="PSUM"))

        identf = c_pool.tile([128, 128], F32)
        make_identity(nc, identf[:])
        dummyf = c_pool.tile([128, 128], F32, tag="dummyf")

        # warm the ACT Sqrt table while the first DMA flies
        warm = c_pool.tile([128, 1], F32, tag="warm")
        nc.vector.memset(warm[:], 1.0)
        warm2 = c_pool.tile([128, 1], F32, tag="warm2")
        nc.scalar.sqrt(warm2[:], warm[:])

        x83s, raws, RIs, rvs, rrs, rivs, nrivs, rvTs = ({}, {}, {}, {}, {}, {}, {}, {})

        def chain(inst):
            tc.chain_iter_dep("pe_order", inst.ins)

        def emit_load(b):
            x8 = x_pool.tile([128, KC * T], F8, tag="x8", name=f"x8_{b}")
            x83 = x8[:].rearrange("p (k t) -> p k t", k=KC)
            src = xT8[b].rearrange("(k p) t -> p k t", p=128)
            for k in range(KC):
                eng = nc.sync if k < KP else nc.scalar
                eng.dma_start(x83[:, k, :], src[:, k, :])
            x83s[b] = x83

        def alloc_batch(b):
            raws[b] = raw_pool.tile([128, ROW_OFF[TT]], BF16, tag="raw",
                                    name=f"raw_{b}")
            RIs[b] = ri_pool.tile([128, T], BF16, tag="RI", name=f"RI_{b}")
            rvs[b] = s_pool.tile([128, TT], F32, tag="rv", name=f"rv_{b}")
            rrs[b] = s_pool.tile([128, TT], F32, tag="rr", name=f"rr_{b}")
            rivs[b] = s_pool.tile([128, TT], F32, tag="riv", name=f"riv_{b}")
            nrivs[b] = s_pool.tile([128, TT], F32, tag="nriv",
                                   name=f"nriv_{b}")

        def chunks_of(m):
            W = T - m * 128
            ch = [(0, min(512, W))]
            if W > 512:
                ch.append((512, W - 512))
            return ch

        def pe_group(b, m):
            # gram row m on raw x8, j-outer; chunk tiles are 1 psum bank;
            # each chunk staged to SBUF bf16 right after its stop matmul.
            x83 = x83s[b]
            n0 = m * 128
            tiles = []
            for ci, (off, w) in enumerate(chunks_of(m)):
                pgt = ps_pool.tile([128, w], F32, tag="pg",
                                   name=f"pg_{b}_{m}_{ci}",
                                   padded_shape=[128, 512])
                tiles.append((pgt, off, w))
            pgs0 = tiles[0][0]
            msl = slice(n0, n0 + 128)
            for j in range(KP):
                for pgt, off, w in tiles:
                    mm = nc.tensor.matmul(
                        pgt[:],
                        x83[:, 2 * j:2 * j + 2, msl],
                        x83[:, 2 * j:2 * j + 2, n0 + off:n0 + off + w],
                        start=(j == 0), stop=(j == KP - 1),
                        perf_mode=DR, skip_group_check=True)
                    chain(mm)
            # ss from the psum diagonal block, before the staging copies
            nc.vector.scalar_tensor_tensor(
                dummyf[:], pgs0[:, 0:128], 1.0, identf[:], ALU.mult,
                ALU.mult, accum_out=rvs[b][:, m:m + 1])
            # stage to SBUF: narrow rows via ACT, wide chunks via DVE
            for pgt, off, w in tiles:
                dst = raws[b][:, ROW_OFF[m] + off:ROW_OFF[m] + off + w]
                if m >= 4:
                    nc.scalar.activation(dst, pgt[:], AF.Copy, bias=0.0,
                                         scale=1.0)
                else:
                    nc.vector.tensor_copy(dst, pgt[:])

        def emit_norm_head(b):
            # once per batch, right after the last extract. Only the DVE
            # reciprocal sits ahead of the PE transpose; sqrt happens
            # after the transpose so the chained PE never waits on ACT.
            nc.vector.reciprocal(rrs[b][:], rvs[b][:])

        def emit_norm_tail(b):
            # PE transpose (of rr = 1/ss) is emitted by the caller; here:
            # row8 = sqrt(rr)^T = +rinv (bf16), DRAM round trip, partition
            # broadcast -> RI (positive). nrivs = -rinv is the per-partition
            # eviction scale; neither is on the PE critical path.
            row8 = s_pool.tile([TT, 128], BF16, tag="row8",
                               name=f"row8_{b}")
            nc.scalar.activation(row8[:], rvTs[b][:], AF.Sqrt, bias=0.0,
                                 scale=1.0)
            nc.sync.dma_start(rowsc[b], row8[:])
            nc.scalar.dma_start(
                RIs[b][:], rowsc[b].unsqueeze(0).to_broadcast((128, T)))
            nc.scalar.activation(rivs[b][:], rrs[b][:], AF.Sqrt, bias=0.0,
                                 scale=1.0)
            nc.scalar.activation(nrivs[b][:], rivs[b][:], AF.Copy, bias=0.0,
                                 scale=-1.0)

        def emit_transpose(b):
            rvT = psT_pool.tile([TT, 128], F32, tag="rvT", name=f"rvT_{b}")
            mmT = nc.tensor.transpose(rvT[:], rrs[b][:], identf[:])
            chain(mmT)
            rvTs[b] = rvT

        def emit_evict(b, m):
            # sc = raw * RI  (all-bf16 SBUF), ob = 1 + sc*rinv_i -> f16
            n0 = m * 128
            W = T - n0
            sc = sc_pool.tile([128, W], BF16, tag="sc", name=f"sc_{b}_{m}",
                              padded_shape=[128, T])
            nc.vector.tensor_tensor(
                sc[:], raws[b][:, ROW_OFF[m]:ROW_OFF[m] + W],
                RIs[b][:, n0:], ALU.mult)
            ob = ob_pool.tile([128, W], F16, tag="ob", name=f"ob_{b}_{m}",
                              padded_shape=[128, T])
            if m >= 4:
                nc.gpsimd.tensor_scalar(ob[:], sc[:],
                                        nrivs[b][:, m:m + 1], 1.0,
                                        ALU.mult, ALU.add)
            else:
                nc.scalar.activation(ob[:], sc[:], AF.Copy, bias=1.0,
                                     scale=nrivs[b][:, m:m + 1])
            eng = nc.sync if m % 2 == 0 else nc.scalar
            eng.dma_start(out[b, n0:n0 + 128, n0:], ob[:])

        # ---- pipeline ----
        emit_load(0)
        if BPC > 1:
            emit_load(1)
        for b in range(BPC):
            alloc_batch(b)
            for m in range(TT):
                if b + 2 < BPC and m == 5:
                    emit_load(b + 2)
                pe_group(b, m)
                if m == TT - 1:
                    emit_norm_head(b)
                if b > 0:
                    if m == 0:
                        emit_transpose(b - 1)
                        emit_norm_tail(b - 1)
                    if 2 <= m <= 5:
                        emit_evict(b - 1, m - 2)
                    elif m == 6:
                        emit_evict(b - 1, 4)
                        emit_evict(b - 1, 5)
                    elif m == 7:
                        emit_evict(b - 1, 6)
                        emit_evict(b - 1, 7)
        # epilogue: last batch's chain + evictions
        b = BPC - 1
        emit_transpose(b)
        emit_norm_tail(b)
        for m in range(TT):
            emit_evict(b, m)

    n = dedup_ldweights(nc)
    assert n > 0, "expected to remove redundant LDWEIGHTS"
    nc.compile()
    return nc


_MIRROR_MASK = None


def host_post(upper_f16):
    """Mirror the upper triangle onto the (unwritten) lower half, f32."""
    global _MIRROR_MASK
    if _MIRROR_MASK is None:
        idx = np.arange(T)
        _MIRROR_MASK = (idx[None, :] >= idx[:, None])[None]  # j >= i
    u = upper_f16.astype(np.float32)
    return np.where(_MIRROR_MASK, u, u.transpose(0, 2, 1))


def host_prep(x):
    x = np.asarray(x)
    x8 = x.astype(ml_dtypes.float8_e4m3)               # [B, T, C]
    xT8 = np.ascontiguousarray(x8.transpose(0, 2, 1))  # [B, C, T]
    return xT8


def run(x, trace=False):
    nc = build()
    xT8 = host_prep(x)
    in_maps = [{"xT8": xT8[i * BPC:(i + 1) * BPC]} for i in range(N_CORES)]
    last_err = None
    for _attempt in range(3):
        try:
            res = run_bass_kernel_spmd(nc, in_maps, list(range(N_CORES)),
                                       trace=trace)
            break
        except Exception as e:  # transient device wedge: retry
            last_err = e
            time.sleep(2.0)
    else:
        raise last_err
    out = np.concatenate([host_post(res.results[i]["out"])
                          for i in range(N_CORES)], axis=0)
    return out, res


def kernel(x):
    out, _ = run(x, trace=False)
    return out
